# revision 26
# baseline (speedup 1.0000x reference)
"""Block-sparse attention Trainium2 kernel (v4, bf16 transposed-AV).

Reference: nn.MultiheadAttention-style block-sparse attention, B=1, L=4096,
D=1024, H=16, head_dim=64, block=128, global blocks {0, 24}.

Sharding: head-parallel across 8 cores (2 heads/core); host sums the 8
partial out-projections. The whole dataflow is bf16 (inputs pre-cast on
host): matmul operands bf16 into f32 PSUM, exps emit bf16, the partial
output is written bf16 and summed in f32 on host.

Attention-value products are computed in transposed form
(outT = v_aug.T @ expT) so every AV matmul has a wide moving dim. Each
head's augmented-V window is [ones | 63 zeros | v.T], which lands the
softmax denominator l in PSUM partition 0 (required by the custom
DVE/gpsimd uops on HW) and the data in the 64-aligned rows 64:128.
Normalization is reciprocal_approx_fast (the exact DVE reciprocal costs
~6ns per free-element on HW) + gpsimd partition_broadcast + one DVE
multiply per 512-wide chunk.

The chunk loop is a 3-stage software pipeline — scores(c) / AV+norm(c-1) /
out-projection(c-2) — so every cross-engine semaphore handoff (~150ns on
HW) has a chunk of slack; the QKV projection quads 1 and 2 are interleaved
into the early pipeline slots to keep the PE at max p-state.
"""

import sys

sys.path.insert(0, "/opt/trn_rl_repo")
import numpy as np

D = 1024
L = 4096
H = 16
HD = 64
NB = 32
GLOB = (0, 24)
P = 128
SCALE = 1.0 / 8.0

PHASES = "full"  # dev knob: "qkv" / "attn1" time sub-phases via mb_phase.py

_CACHE = {}


def _build_nc(reps=1):
    import contextlib

    import concourse.mybir as mybir
    import concourse.tile as tile
    from concourse import bacc
    from concourse.masks import make_identity

    f32 = mybir.dt.float32
    f32r = mybir.dt.float32r
    bf16 = mybir.dt.bfloat16
    Act = mybir.ActivationFunctionType
    AluMult = mybir.AluOpType.mult

    nc = bacc.Bacc("TRN2", target_bir_lowering=False, debug=False, num_devices=8)
    xT = nc.dram_tensor("xT", [D, L], bf16, kind="ExternalInput")
    wq = nc.dram_tensor("wq", [P, D], bf16, kind="ExternalInput")
    wk = nc.dram_tensor("wk", [P, D], bf16, kind="ExternalInput")
    wv = nc.dram_tensor("wv", [P, D], bf16, kind="ExternalInput")
    wo = nc.dram_tensor("wo", [P, D], bf16, kind="ExternalInput")
    bq = nc.dram_tensor("bq", [P, 1], f32, kind="ExternalInput")
    bk = nc.dram_tensor("bk", [P, 1], f32, kind="ExternalInput")
    out = nc.dram_tensor("out", [L, D], bf16, kind="ExternalOutput")

    with tile.TileContext(nc) as tc:
        with (
            tc.tile_pool(name="const", bufs=1) as constp,
            tc.tile_pool(name="stream", bufs=3) as streamp,
            tc.tile_pool(name="expb", bufs=6) as expp,
            tc.tile_pool(name="small", bufs=4) as smallp,
            tc.tile_pool(name="ps_big", bufs=2, space="PSUM") as ps_big,
            tc.tile_pool(name="ps_med", bufs=2, space="PSUM") as ps_med,
            tc.tile_pool(name="ps_av", bufs=2, space="PSUM") as ps_av,
        ):
            # ---------- constants / persistent buffers
            ident = constp.tile([P, P], bf16, tag="ident")
            make_identity(nc, ident[:])
            ones_col = constp.tile([P, 1], bf16, tag="ones")
            nc.vector.memset(ones_col[:], 1.0)

            wq_r = constp.tile([P, D], bf16, tag="wq_r")
            wk_r = constp.tile([P, D], bf16, tag="wk_r")
            wv_r = constp.tile([P, D], bf16, tag="wv_r")
            wo_r = constp.tile([P, D], bf16, tag="wo_r")
            for dram, tr in ((wq, wq_r), (wk, wk_r), (wv, wv_r), (wo, wo_r)):
                nc.sync.dma_start(tr[:], dram[:])
            bq_t = constp.tile([P, 1], f32, tag="bq")
            bk_t = constp.tile([P, 1], f32, tag="bk")
            nc.sync.dma_start(bq_t[:], bq[:])
            nc.sync.dma_start(bk_t[:], bk[:])

            qT = constp.tile([P, L], bf16, tag="qT")
            kT = constp.tile([P, L], bf16, tag="kT")
            vTf = constp.tile([P, L], bf16, tag="vTf")
            # augmented-V blocks: per block 2 windows of 128 cols, one per
            # head: [ones | 63 zeros | v.T (64)] — so each head's AV psum gets
            # l in row 0 (partition 0, where the custom DVE/gpsimd uops need
            # it) and data in the 64-aligned rows 64:128. Pad columns cost
            # nothing on PE (matmul cost is moving-dim rows only).
            vn = constp.tile([P, NB * 256], bf16, tag="vn")
            nc.vector.memset(vn[:], 0.0)
            qg = constp.tile([P, 256], bf16, tag="qg")
            gout = constp.tile([P, 256], bf16, tag="gout")
            for _b in range(NB):
                nc.vector.tensor_copy(vn[:, _b * 256:_b * 256 + 1], ones_col[:])
                nc.vector.tensor_copy(vn[:, _b * 256 + 128:_b * 256 + 129], ones_col[:])

            loop_ctx = tc.For_i(0, reps, 1) if reps > 1 else contextlib.nullcontext()
            with loop_ctx:
                _body(nc, tc, mybir, Act, f32, f32r, bf16, AluMult, locals())

    nc.compile()
    return nc


def _body(nc, tc, mybir, Act, f32, f32r, bf16, AluMult, env):
    constp = env["constp"]; streamp = env["streamp"]; expp = env["expp"]; smallp = env["smallp"]
    ps_big = env["ps_big"]; ps_med = env["ps_med"]; ps_av = env["ps_av"]
    ident = env["ident"]; ones_col = env["ones_col"]
    wq_r = env["wq_r"]; wk_r = env["wk_r"]; wv_r = env["wv_r"]; wo_r = env["wo_r"]
    bq_t = env["bq_t"]; bk_t = env["bk_t"]
    qT = env["qT"]; kT = env["kT"]; vTf = env["vTf"]; vn = env["vn"]
    qg = env["qg"]; gout = env["gout"]
    xT = env["xT"]; out = env["out"]
    AluAdd = mybir.AluOpType.add

    # ---------- phase A: qkv projections + fused v-transpose.
    # Split per 512-token sub-half so the PE work (3 psum chains + 4
    # transposes ≈ 5.6us) can be spread across chunk slots.
    def do_quad_dma(quad):
        xrs = []
        for kt in range(8):
            xraw = streamp.tile([P, 1024], bf16, tag="xraw", bufs=16,
                                name=f"xraw{quad}_{kt}")
            nc.sync.dma_start(
                xraw[:], xT[kt * P:(kt + 1) * P, quad * 1024:(quad + 1) * 1024]
            )
            xrs.append(xraw)
        return xrs

    def do_quad_sub(quad, sub, xrs):
        n = quad * 2 + sub
        sl = slice(n * 512, (n + 1) * 512)
        for wt, dest, bias in (
            (wq_r, qT, bq_t),
            (wk_r, kT, bk_t),
            (wv_r, vTf, None),
        ):
            pp = ps_big.tile([P, 512], f32, tag="psbig")
            for kt in range(8):
                nc.tensor.matmul(
                    pp[:], wt[:, kt * P:(kt + 1) * P],
                    xrs[kt][:, sub * 512:(sub + 1) * 512],
                    start=kt == 0, stop=kt == 7,
                )
            if bias is not None:
                nc.scalar.activation(dest[:, sl], pp[:], Act.Identity, bias=bias[:])
            else:
                nc.vector.tensor_copy(dest[:, sl], pp[:])
        for b in range(8 * quad + 4 * sub, 8 * quad + 4 * sub + 4):
            pst = ps_av.tile([P, P], bf16, tag="psav", name=f"pst{b}")
            nc.tensor.transpose(pst[:], vTf[:, b * P:(b + 1) * P], ident[:])
            base = b * 256
            # one strided copy: pst halves -> the two head windows' rows 64:128
            dst = vn[:, base:base + 256].rearrange("p (a b) -> p a b", a=2, b=128)
            src = pst[:, 0:128].rearrange("p (a b) -> p a b", a=2, b=64)
            nc.vector.tensor_copy(dst[:, :, 64:128], src)

    def do_quad(quad):
        xrs = do_quad_dma(quad)
        do_quad_sub(quad, 0, xrs)
        do_quad_sub(quad, 1, xrs)

    do_quad(0)
    do_quad(3)
    # stage global-q columns (available after quads 0 and 3)
    nc.vector.tensor_copy(qg[:, 0:128], qT[:, 0:128])
    nc.vector.tensor_copy(qg[:, 128:256], qT[:, GLOB[1] * P:(GLOB[1] + 1) * P])

    def vslice(blk, h):
        return vn[:, blk * 256 + h * 128: blk * 256 + (h + 1) * 128]

    def norm_prep(src, lo, hi, l_in_sbuf=False):
        # src [128, W+]: row 0 = l (partition 0 — where the custom DVE/gpsimd
        # uops need it on HW), rows 64:128 = unnormalized outT.
        # Returns [64, W] broadcast of 1/l. reciprocal_approx_fast: the exact
        # DVE reciprocal costs ~6ns per free-element on HW (~2.9us per row),
        # the approx one ~0.7ns; softmax denominators don't need the bits.
        W = hi - lo
        linv = smallp.tile([1, 512], f32, tag="linv")
        with nc.allow_low_precision(reason="softmax denom tolerates approx"):
            nc.vector.reciprocal_approx_fast(linv[0:1, 0:W], src[0:1, lo:hi])
        bsb = smallp.tile([64, 512], f32, tag="bsb")
        nc.gpsimd.partition_broadcast(bsb[0:64, 0:W], linv[0:1, 0:W])
        return bsb

    def norm_mult(src, bsb, lo, hi, dest):
        W = hi - lo
        nc.vector.tensor_tensor(
            dest, src[64:128, lo:hi], bsb[0:64, 0:W], AluMult
        )

    def normalize_emit(src, h, lo, hi, dest, l_in_sbuf=False):
        norm_mult(src, norm_prep(src, lo, hi, l_in_sbuf), lo, hi, dest)

    # ---------- global qtiles (0 and 24): attend to all 32 blocks.
    # Folded into the chunk sweep: each chunk slot also scores the 256 global
    # queries against its own 4 key blocks, AV-accumulates them in a chunk-
    # local PSUM tile, and adds that into a persistent SBUF accumulator
    # (gacc). This spreads the 32 global exps evenly across the kernel
    # instead of an ACT-bound stretch, and pins no PSUM slots.
    gacc = constp.tile([P, 512], f32, tag="gacc")

    def do_gq(c, first=False):
        hsl = (slice(0, 64), slice(64, 128))
        psq = {h: ps_med.tile([P, 1024], f32, tag="psmed",
                              name=f"gq{c}_{h}") for h in (0, 1)}
        for i in range(4):
            kb = 4 * c + i
            for h in (0, 1):
                nc.tensor.matmul(
                    psq[h][:, i * 256:(i + 1) * 256],
                    kT[hsl[h], kb * P:(kb + 1) * P], qg[hsl[h], :],
                    start=True, stop=True, skip_group_check=True,
                )
        egq = {}
        for h in (0, 1):
            eq = expp.tile([P, 1024], bf16, tag="gexp")
            nc.scalar.activation(eq[:], psq[h][:], Act.Exp, scale=SCALE)
            egq[h] = eq
        for h in (0, 1):
            pgc = ps_av.tile([P, 256], f32, tag="psav", name=f"pgc{c}_{h}")
            for i in range(4):
                nc.tensor.matmul(
                    pgc[:], vslice(4 * c + i, h),
                    egq[h][:, i * 256:(i + 1) * 256],
                    start=i == 0, stop=i == 3,
                )
            gsl = gacc[:, h * 256:(h + 1) * 256]
            if first:
                nc.vector.tensor_copy(gsl, pgc[:])
            else:
                nc.vector.tensor_tensor(gsl, gsl, pgc[:], AluAdd)

    def do_global_close():
        # stage gacc back into PSUM so normalization uses the exact partition
        # placement the custom reciprocal/broadcast uops are proven with on
        # HW (l in PSUM partition 0) — the SBUF-sourced variant miscomputed
        # on hardware.
        pgf = ps_av.tile([P, 512], f32, tag="psav", name="pgf")
        nc.vector.tensor_copy(pgf[:], gacc[:])
        for h in (0, 1):
            normalize_emit(pgf[:, h * 256:(h + 1) * 256], h, 0, 256,
                           gout[h * 64:(h + 1) * 64, :])

    # ---------- chunk pipeline: S(c) scores+exps, A(c) AV+normalize,
    # O(c) out-projection. Emitted as S(c) / A(c-1) / O(c-2) so every
    # cross-engine dependency has ~a full chunk of slack to absorb the
    # ~150ns semaphore handoff latency of the hardware.
    otrs, egss, edss = {}, {}, {}

    def chunk_info(c):
        glob_in_chunk = [g for g in GLOB if g // 4 == c]
        lo = 128 if glob_in_chunk else 0
        qts = [4 * c + i for i in range(4) if (4 * c + i) not in GLOB]
        return glob_in_chunk, lo, qts

    def do_scores(c, diag_in_med=False):
        # h0/h1 matmul pairs are emitted adjacently: their stationaries sit at
        # base partitions 0 and 64, so the PE runs them concurrently in
        # disjoint row-groups (64-contract packing, ~2x on score matmuls).
        # diag_in_med: chunks scored while the global pg accumulators pin two
        # ps_av slots take their diag psum from ps_med instead.
        _, _, qts = chunk_info(c)
        nq = len(qts)
        hsl = (slice(0, 64), slice(64, 128))
        # both global key blocks' scores land in one 2-bank tile per head so
        # a single 1024-wide exp drains them (the ~290ns fixed ACT cost per
        # instruction is the scarce resource, not elements).
        psgs = {h: ps_med.tile([P, 1024], f32, tag="psmed",
                               name=f"psg{c}_{h}") for h in (0, 1)}
        for gi, g in enumerate(GLOB):
            for h in (0, 1):
                nc.tensor.matmul(
                    psgs[h][:, gi * 512:(gi + 1) * 512],
                    kT[hsl[h], g * P:(g + 1) * P],
                    qT[hsl[h], c * 512:(c + 1) * 512],
                    start=True, stop=True, skip_group_check=True,
                )
        for h in (0, 1):
            eg = expp.tile([P, 1024], bf16, tag="exp", bufs=12)
            nc.scalar.activation(eg[:], psgs[h][:], Act.Exp, scale=SCALE)
            egss[c, h] = eg
        psds = {h: ps_av.tile([P, 512], f32, tag="psav", name=f"psd{c}_{h}")
                for h in (0, 1)}
        for idx, j in enumerate(qts):
            for h in (0, 1):
                nc.tensor.matmul(
                    psds[h][:, idx * P:(idx + 1) * P],
                    kT[hsl[h], j * P:(j + 1) * P], qT[hsl[h], j * P:(j + 1) * P],
                    start=True, stop=True, skip_group_check=True,
                )
        for h in (0, 1):
            ed = expp.tile([P, 512], bf16, tag="exp", bufs=12)
            nc.scalar.activation(
                ed[:, 0:nq * P], psds[h][:, 0:nq * P], Act.Exp, scale=SCALE
            )
            edss[c, h] = ed

    def do_av(c):
        glob_in_chunk, lo, qts = chunk_info(c)
        otr = smallp.tile([P, 512], bf16, tag="otr", name=f"otr{c}")
        otrs[c] = otr
        if glob_in_chunk:
            g = glob_in_chunk[0]
            gq_col = 0 if g == 0 else 128
            nc.vector.tensor_copy(otr[:, 0:128], gout[:, gq_col:gq_col + 128])
        pcs = {}
        for h in (0, 1):
            pc = ps_av.tile([P, 512], f32, tag="psav")
            pcs[h] = pc
            nc.tensor.matmul(pc[:, lo:512], vslice(GLOB[0], h),
                             egss[c, h][:, lo:512],
                             start=True, stop=False)
            nc.tensor.matmul(pc[:, lo:512], vslice(GLOB[1], h),
                             egss[c, h][:, 512 + lo:1024],
                             start=False, stop=True)
            for idx, j in enumerate(qts):
                off = (j - 4 * c) * P
                nc.tensor.matmul(pc[:, off:off + P], vslice(j, h),
                                 edss[c, h][:, idx * P:(idx + 1) * P],
                                 start=False, stop=True,
                                 skip_group_check=True)  # sub-region accumulate
        # both l-extracts+broadcasts first, then both divides: DVE never
        # sits head-of-line waiting for a Pool broadcast round-trip.
        bsbs = {h: norm_prep(pcs[h], lo, 512) for h in (0, 1)}
        for h in (0, 1):
            norm_mult(pcs[h], bsbs[h], lo, 512, otr[h * 64:(h + 1) * 64, lo:512])

    def do_outproj(c, act_frac=2):
        # drain copies go (8-act_frac):act_frac DVE:ACT per chunk — ACT is
        # saturated by exps mid-kernel (act_frac=2); the tail chunks run when
        # exps are done, so they split evenly (act_frac=4).
        otr = otrs[c]
        n_copy = 0
        for t in range(4):
            j = 4 * c + t
            osb = streamp.tile([P, D], bf16, tag="osb")
            for half in (0, 1):
                pso = ps_big.tile([P, 512], f32, tag="psbig")
                nc.tensor.matmul(
                    pso[:], otr[:, t * P:(t + 1) * P],
                    wo_r[:, half * 512:(half + 1) * 512],
                    start=True, stop=True,
                )
                sl = slice(half * 512, (half + 1) * 512)
                if n_copy % (8 // act_frac) == 0:
                    nc.scalar.activation(osb[:, sl], pso[:], Act.Identity)
                else:
                    nc.vector.tensor_copy(osb[:, sl], pso[:])
                n_copy += 1
            nc.gpsimd.dma_start(out[j * P:(j + 1) * P, :], osb[:])

    if PHASES == "qkv":
        do_quad(1)
        do_quad(2)
        return
    if PHASES == "attn1":
        do_quad(1); do_quad(2)
        do_scores(1)
        do_av(1)
        do_outproj(1)
        return
    # wavefront: S one chunk ahead of A, two ahead of O. Within each slot the
    # consumer phases (A, O) are emitted BEFORE S(c): a score matmul stalled
    # on its PSUM rotation (waiting for an exp) then sits at the END of the
    # PE FIFO instead of head-of-line blocking the already-runnable AV and
    # out-projection matmuls. Quads 1/2 fill the early slots; do_global is
    # split around outproj(4) so its ACT-bound exp stretch has PE company.
    do_scores(1); do_gq(1, first=True)
    xrs1 = do_quad_dma(1)
    do_quad_sub(1, 0, xrs1)
    do_av(1); do_scores(7); do_gq(7)
    do_quad_sub(1, 1, xrs1)
    do_av(7); do_outproj(1); do_scores(2); do_gq(2)
    xrs2 = do_quad_dma(2)
    do_quad_sub(2, 0, xrs2)
    do_av(2); do_outproj(7); do_scores(3); do_gq(3)
    do_quad_sub(2, 1, xrs2)
    do_av(3); do_outproj(2); do_scores(4); do_gq(4)
    do_av(4); do_outproj(3); do_scores(5); do_gq(5)
    do_av(5); do_outproj(4); do_scores(0); do_gq(0)
    do_scores(6); do_gq(6)
    do_global_close()
    do_outproj(5)
    do_av(0); do_av(6)
    # interleave the two final out-projections tile-by-tile so each chunk's
    # drain copies overlap the other's matmuls
    otr0, otr6 = otrs[0], otrs[6]
    n_copy = 0
    for t in range(4):
        for c, otr in ((0, otr0), (6, otr6)):
            j = 4 * c + t
            osb = streamp.tile([P, D], bf16, tag="osb", name=f"osbt{c}_{t}")
            for half in (0, 1):
                pso = ps_big.tile([P, 512], f32, tag="psbig")
                nc.tensor.matmul(
                    pso[:], otr[:, t * P:(t + 1) * P],
                    wo_r[:, half * 512:(half + 1) * 512],
                    start=True, stop=True,
                )
                sl = slice(half * 512, (half + 1) * 512)
                if n_copy % 2 == 0:
                    nc.scalar.activation(osb[:, sl], pso[:], Act.Identity)
                else:
                    nc.vector.tensor_copy(osb[:, sl], pso[:])
                n_copy += 1
            nc.gpsimd.dma_start(out[j * P:(j + 1) * P, :], osb[:])


def _get_nc(reps=1):
    key = ("nc", reps)
    if key not in _CACHE:
        _CACHE[key] = _build_nc(reps)
    return _CACHE[key]


def _bf16(a):
    import ml_dtypes

    return np.asarray(a, dtype=np.float32).astype(ml_dtypes.bfloat16)


def _prep_inputs(x, w_qkv, b_qkv):
    x2 = np.asarray(x, dtype=np.float32).reshape(L, D)
    xT = _bf16(np.ascontiguousarray(x2.T))
    w_qkv = np.asarray(w_qkv, dtype=np.float32)
    b_qkv = np.asarray(b_qkv, dtype=np.float32)

    def tile_w(w_slice):
        wt = w_slice.T
        return _bf16(np.ascontiguousarray(
            wt.reshape(8, P, P).transpose(1, 0, 2).reshape(P, D)
        ))

    maps = []
    for c in range(8):
        a = 2 * c * HD
        b = a + 2 * HD
        maps.append({
            "xT": xT,
            "wq": tile_w(w_qkv[a:b, :]),
            "wk": tile_w(w_qkv[D + a:D + b, :]),
            "wv": tile_w(w_qkv[2 * D + a:2 * D + b, :]),
            "bq": np.ascontiguousarray(b_qkv[a:b].reshape(P, 1)),
            "bk": np.ascontiguousarray(b_qkv[D + a:D + b].reshape(P, 1)),
        })
    return maps


def kernel(x, w_qkv, b_qkv, w_out, b_out):
    from concourse.bass_utils import run_bass_kernel_spmd

    x = np.asarray(x, dtype=np.float32)
    w_qkv = np.asarray(w_qkv, dtype=np.float32)
    b_qkv = np.asarray(b_qkv, dtype=np.float32)
    w_out = np.asarray(w_out, dtype=np.float32)
    b_out = np.asarray(b_out, dtype=np.float32)

    nc = _get_nc()
    maps = _prep_inputs(x, w_qkv, b_qkv)
    for c in range(8):
        a = 2 * c * HD
        b = a + 2 * HD
        maps[c]["wo"] = _bf16(np.ascontiguousarray(w_out[:, a:b].T))

    res = run_bass_kernel_spmd(nc, maps, core_ids=list(range(8)))

    total = res.results[0]["out"].astype(np.float32)
    for c in range(1, 8):
        total += res.results[c]["out"].astype(np.float32)
    const_row = b_qkv[2 * D:3 * D] @ w_out.T + b_out
    total += const_row[None, :]
    return total.reshape(x.shape).astype(np.float32)



# revision 31
# speedup vs baseline: 1.0899x; 1.0899x over previous
"""Block-sparse attention Trainium2 kernel (v4, bf16 transposed-AV).

Reference: nn.MultiheadAttention-style block-sparse attention, B=1, L=4096,
D=1024, H=16, head_dim=64, block=128, global blocks {0, 24}.

Sharding: head-parallel across 8 cores (2 heads/core); host sums the 8
partial out-projections. The whole dataflow is bf16 (inputs pre-cast on
host): matmul operands bf16 into f32 PSUM, exps emit bf16, the partial
output is written bf16 and summed in f32 on host.

Attention-value products are computed in transposed form
(outT = v_aug.T @ expT) so every AV matmul has a wide moving dim. Each
head's augmented-V window is [ones | 63 zeros | v.T], which lands the
softmax denominator l in PSUM partition 0 (required by the custom
DVE/gpsimd uops on HW) and the data in the 64-aligned rows 64:128.
Normalization is reciprocal_approx_fast (the exact DVE reciprocal costs
~6ns per free-element on HW) + gpsimd partition_broadcast + one DVE
multiply per 512-wide chunk.

The chunk loop is a 3-stage software pipeline — scores(c) / AV+norm(c-1) /
out-projection(c-2) — so every cross-engine semaphore handoff (~150ns on
HW) has a chunk of slack; the QKV projection quads 1 and 2 are interleaved
into the early pipeline slots to keep the PE at max p-state.
"""

import sys

sys.path.insert(0, "/opt/trn_rl_repo")
import numpy as np

D = 1024
L = 4096
H = 16
HD = 64
NB = 32
GLOB = (0, 24)
P = 128
SCALE = 1.0 / 8.0

PHASES = "full"  # dev knob: "qkv" / "attn1" time sub-phases via mb_phase.py

_CACHE = {}


def _build_nc(reps=1):
    import contextlib

    import concourse.mybir as mybir
    import concourse.tile as tile
    from concourse import bacc
    from concourse.masks import make_identity

    f32 = mybir.dt.float32
    f32r = mybir.dt.float32r
    bf16 = mybir.dt.bfloat16
    Act = mybir.ActivationFunctionType
    AluMult = mybir.AluOpType.mult

    nc = bacc.Bacc("TRN2", target_bir_lowering=False, debug=False, num_devices=8)
    xT = nc.dram_tensor("xT", [D, L], bf16, kind="ExternalInput")
    wq = nc.dram_tensor("wq", [P, D], bf16, kind="ExternalInput")
    wk = nc.dram_tensor("wk", [P, D], bf16, kind="ExternalInput")
    wv = nc.dram_tensor("wv", [P, D], bf16, kind="ExternalInput")
    wo = nc.dram_tensor("wo", [P, D], bf16, kind="ExternalInput")
    bq = nc.dram_tensor("bq", [P, 1], f32, kind="ExternalInput")
    bk = nc.dram_tensor("bk", [P, 1], f32, kind="ExternalInput")
    out = nc.dram_tensor("out", [L, D], bf16, kind="ExternalOutput")

    with tile.TileContext(nc) as tc:
        with (
            tc.tile_pool(name="const", bufs=1) as constp,
            tc.tile_pool(name="stream", bufs=3) as streamp,
            tc.tile_pool(name="expb", bufs=6) as expp,
            tc.tile_pool(name="small", bufs=4) as smallp,
            tc.tile_pool(name="ps_big", bufs=2, space="PSUM") as ps_big,
            tc.tile_pool(name="ps_med", bufs=3, space="PSUM") as ps_med,
            tc.tile_pool(name="ps_av", bufs=3, space="PSUM") as ps_av,
        ):
            # ---------- constants / persistent buffers
            ident = constp.tile([P, P], bf16, tag="ident")
            make_identity(nc, ident[:])
            ones_col = constp.tile([P, 1], bf16, tag="ones")
            nc.vector.memset(ones_col[:], 1.0)

            wq_r = constp.tile([P, D], bf16, tag="wq_r")
            wk_r = constp.tile([P, D], bf16, tag="wk_r")
            wv_r = constp.tile([P, D], bf16, tag="wv_r")
            wo_r = constp.tile([P, D], bf16, tag="wo_r")
            for dram, tr in ((wq, wq_r), (wk, wk_r), (wv, wv_r), (wo, wo_r)):
                nc.sync.dma_start(tr[:], dram[:])
            bq_t = constp.tile([P, 1], f32, tag="bq")
            bk_t = constp.tile([P, 1], f32, tag="bk")
            nc.sync.dma_start(bq_t[:], bq[:])
            nc.sync.dma_start(bk_t[:], bk[:])

            qT = constp.tile([P, L], bf16, tag="qT")
            kT = constp.tile([P, L], bf16, tag="kT")
            vTf = constp.tile([P, L], bf16, tag="vTf")
            # augmented-V blocks: per block 2 windows of 128 cols, one per
            # head: [ones | 63 zeros | v.T (64)] — so each head's AV psum gets
            # l in row 0 (partition 0, where the custom DVE/gpsimd uops need
            # it) and data in the 64-aligned rows 64:128. Pad columns cost
            # nothing on PE (matmul cost is moving-dim rows only).
            vn = constp.tile([P, NB * 256], bf16, tag="vn")
            nc.vector.memset(vn[:], 0.0)
            qg = constp.tile([P, 256], bf16, tag="qg")
            gout = constp.tile([P, 256], bf16, tag="gout")
            for _b in range(NB):
                nc.vector.tensor_copy(vn[:, _b * 256:_b * 256 + 1], ones_col[:])
                nc.vector.tensor_copy(vn[:, _b * 256 + 128:_b * 256 + 129], ones_col[:])

            loop_ctx = tc.For_i(0, reps, 1) if reps > 1 else contextlib.nullcontext()
            with loop_ctx:
                _body(nc, tc, mybir, Act, f32, f32r, bf16, AluMult, locals())

    nc.compile()
    return nc


def _body(nc, tc, mybir, Act, f32, f32r, bf16, AluMult, env):
    constp = env["constp"]; streamp = env["streamp"]; expp = env["expp"]; smallp = env["smallp"]
    ps_big = env["ps_big"]; ps_med = env["ps_med"]; ps_av = env["ps_av"]
    ident = env["ident"]; ones_col = env["ones_col"]
    wq_r = env["wq_r"]; wk_r = env["wk_r"]; wv_r = env["wv_r"]; wo_r = env["wo_r"]
    bq_t = env["bq_t"]; bk_t = env["bk_t"]
    qT = env["qT"]; kT = env["kT"]; vTf = env["vTf"]; vn = env["vn"]
    qg = env["qg"]; gout = env["gout"]
    xT = env["xT"]; out = env["out"]
    AluAdd = mybir.AluOpType.add

    # ---------- phase A: qkv projections + fused v-transpose.
    # Split per 512-token sub-half so the PE work (3 psum chains + 4
    # transposes ≈ 5.6us) can be spread across chunk slots.
    def do_quad_dma(quad):
        xrs = []
        for kt in range(8):
            xraw = streamp.tile([P, 1024], bf16, tag="xraw", bufs=16,
                                name=f"xraw{quad}_{kt}")
            nc.sync.dma_start(
                xraw[:], xT[kt * P:(kt + 1) * P, quad * 1024:(quad + 1) * 1024]
            )
            xrs.append(xraw)
        return xrs

    def do_quad_sub(quad, sub, xrs):
        n = quad * 2 + sub
        sl = slice(n * 512, (n + 1) * 512)
        for wt, dest, bias in (
            (wq_r, qT, bq_t),
            (wk_r, kT, bk_t),
            (wv_r, vTf, None),
        ):
            pp = ps_big.tile([P, 512], f32, tag="psbig")
            for kt in range(8):
                nc.tensor.matmul(
                    pp[:], wt[:, kt * P:(kt + 1) * P],
                    xrs[kt][:, sub * 512:(sub + 1) * 512],
                    start=kt == 0, stop=kt == 7,
                )
            if bias is not None:
                nc.scalar.activation(dest[:, sl], pp[:], Act.Identity, bias=bias[:])
            else:
                nc.vector.tensor_copy(dest[:, sl], pp[:])
        for b in range(8 * quad + 4 * sub, 8 * quad + 4 * sub + 4):
            pst = ps_av.tile([P, P], bf16, tag="psav", name=f"pst{b}")
            nc.tensor.transpose(pst[:], vTf[:, b * P:(b + 1) * P], ident[:])
            base = b * 256
            # one strided copy: pst halves -> the two head windows' rows 64:128
            dst = vn[:, base:base + 256].rearrange("p (a b) -> p a b", a=2, b=128)
            src = pst[:, 0:128].rearrange("p (a b) -> p a b", a=2, b=64)
            nc.vector.tensor_copy(dst[:, :, 64:128], src)

    def do_quad(quad):
        xrs = do_quad_dma(quad)
        do_quad_sub(quad, 0, xrs)
        do_quad_sub(quad, 1, xrs)

    do_quad(0)
    do_quad(3)
    # stage global-q columns (available after quads 0 and 3)
    nc.vector.tensor_copy(qg[:, 0:128], qT[:, 0:128])
    nc.vector.tensor_copy(qg[:, 128:256], qT[:, GLOB[1] * P:(GLOB[1] + 1) * P])

    def vslice(blk, h):
        return vn[:, blk * 256 + h * 128: blk * 256 + (h + 1) * 128]

    def norm_prep(src, lo, hi, l_in_sbuf=False):
        # src [128, W+]: row 0 = l (partition 0 — where the custom DVE/gpsimd
        # uops need it on HW), rows 64:128 = unnormalized outT.
        # Returns [64, W] broadcast of 1/l. reciprocal_approx_fast: the exact
        # DVE reciprocal costs ~6ns per free-element on HW (~2.9us per row),
        # the approx one ~0.7ns; softmax denominators don't need the bits.
        W = hi - lo
        linv = smallp.tile([1, 512], f32, tag="linv")
        with nc.allow_low_precision(reason="softmax denom tolerates approx"):
            nc.vector.reciprocal_approx_fast(linv[0:1, 0:W], src[0:1, lo:hi])
        bsb = smallp.tile([64, 512], f32, tag="bsb")
        nc.gpsimd.partition_broadcast(bsb[0:64, 0:W], linv[0:1, 0:W])
        return bsb

    def norm_mult(src, bsb, lo, hi, dest):
        W = hi - lo
        nc.vector.tensor_tensor(
            dest, src[64:128, lo:hi], bsb[0:64, 0:W], AluMult
        )

    def normalize_emit(src, h, lo, hi, dest, l_in_sbuf=False):
        norm_mult(src, norm_prep(src, lo, hi, l_in_sbuf), lo, hi, dest)

    # ---------- global qtiles (0 and 24): attend to all 32 blocks.
    # pg comes from the ps_av pool: during do_global no chunk AVs run, so the
    # pool's rotation has slack for one long-lived accumulator per head.
    # Emitted in ranges so other-phase PE work can be interleaved between the
    # ACT-bound exp stretches; h0/h1 score matmuls are adjacent for row-group
    # packing.
    def do_global_open():
        return {h: ps_av.tile([P, 256], f32, tag="psav", name=f"pg{h}")
                for h in (0, 1)}

    def do_global_range(pgs, lo, hi):
        hsl = (slice(0, 64), slice(64, 128))
        for kb2 in range(lo, hi):
            psgs = {h: ps_med.tile([P, 512], f32, tag="psmed",
                                   name=f"psg{kb2}_{h}") for h in (0, 1)}
            for half in (0, 1):
                kb = 2 * kb2 + half
                for h in (0, 1):
                    nc.tensor.matmul(
                        psgs[h][:, half * 256:(half + 1) * 256],
                        kT[hsl[h], kb * P:(kb + 1) * P], qg[hsl[h], :],
                        start=True, stop=True,
                    )
            egs = {}
            for h in (0, 1):
                eg = expp.tile([P, 512], bf16, tag="gexp")
                nc.scalar.activation(eg[:], psgs[h][:], Act.Exp, scale=SCALE)
                egs[h] = eg
            for h in (0, 1):
                for half in (0, 1):
                    kb = 2 * kb2 + half
                    nc.tensor.matmul(
                        pgs[h][:], vslice(kb, h),
                        egs[h][:, half * 256:(half + 1) * 256],
                        start=kb == 0, stop=kb == NB - 1,
                    )

    def do_global_close(pgs):
        for h in (0, 1):
            normalize_emit(pgs[h], h, 0, 256, gout[h * 64:(h + 1) * 64, :])

    # ---------- chunk pipeline: S(c) scores+exps, A(c) AV+normalize,
    # O(c) out-projection. Emitted as S(c) / A(c-1) / O(c-2) so every
    # cross-engine dependency has ~a full chunk of slack to absorb the
    # ~150ns semaphore handoff latency of the hardware.
    otrs, egss, edss = {}, {}, {}

    def chunk_info(c):
        glob_in_chunk = [g for g in GLOB if g // 4 == c]
        lo = 128 if glob_in_chunk else 0
        qts = [4 * c + i for i in range(4) if (4 * c + i) not in GLOB]
        return glob_in_chunk, lo, qts

    def do_scores(c, diag_in_med=False):
        # h0/h1 matmul pairs are emitted adjacently: their stationaries sit at
        # base partitions 0 and 64, so the PE runs them concurrently in
        # disjoint row-groups (64-contract packing, ~2x on score matmuls).
        # diag_in_med: chunks scored while the global pg accumulators pin two
        # ps_av slots take their diag psum from ps_med instead.
        _, _, qts = chunk_info(c)
        nq = len(qts)
        hsl = (slice(0, 64), slice(64, 128))
        for g in GLOB:
            psgs = {}
            for h in (0, 1):
                psg = ps_med.tile([P, 512], f32, tag="psmed")
                nc.tensor.matmul(
                    psg[:], kT[hsl[h], g * P:(g + 1) * P],
                    qT[hsl[h], c * 512:(c + 1) * 512],
                    start=True, stop=True,
                )
                psgs[h] = psg
            for h in (0, 1):
                eg = expp.tile([P, 512], bf16, tag="exp", bufs=12)
                nc.scalar.activation(eg[:], psgs[h][:], Act.Exp, scale=SCALE)
                egss[c, g, h] = eg
        dpool, dtag = (ps_med, "psmed") if diag_in_med else (ps_av, "psav")
        psds = {h: dpool.tile([P, 512], f32, tag=dtag, name=f"psd{c}_{h}")
                for h in (0, 1)}
        for idx, j in enumerate(qts):
            for h in (0, 1):
                nc.tensor.matmul(
                    psds[h][:, idx * P:(idx + 1) * P],
                    kT[hsl[h], j * P:(j + 1) * P], qT[hsl[h], j * P:(j + 1) * P],
                    start=True, stop=True, skip_group_check=True,
                )
        for h in (0, 1):
            ed = expp.tile([P, 512], bf16, tag="exp", bufs=12)
            nc.scalar.activation(
                ed[:, 0:nq * P], psds[h][:, 0:nq * P], Act.Exp, scale=SCALE
            )
            edss[c, h] = ed

    def do_av(c):
        glob_in_chunk, lo, qts = chunk_info(c)
        otr = smallp.tile([P, 512], bf16, tag="otr", name=f"otr{c}")
        otrs[c] = otr
        if glob_in_chunk:
            g = glob_in_chunk[0]
            gq_col = 0 if g == 0 else 128
            nc.vector.tensor_copy(otr[:, 0:128], gout[:, gq_col:gq_col + 128])
        pcs = {}
        for h in (0, 1):
            pc = ps_av.tile([P, 512], f32, tag="psav")
            pcs[h] = pc
            nc.tensor.matmul(pc[:, lo:512], vslice(GLOB[0], h),
                             egss[c, GLOB[0], h][:, lo:512],
                             start=True, stop=False)
            nc.tensor.matmul(pc[:, lo:512], vslice(GLOB[1], h),
                             egss[c, GLOB[1], h][:, lo:512],
                             start=False, stop=True)
            for idx, j in enumerate(qts):
                off = (j - 4 * c) * P
                nc.tensor.matmul(pc[:, off:off + P], vslice(j, h),
                                 edss[c, h][:, idx * P:(idx + 1) * P],
                                 start=False, stop=True,
                                 skip_group_check=True)  # sub-region accumulate
        # both l-extracts+broadcasts first, then both divides: DVE never
        # sits head-of-line waiting for a Pool broadcast round-trip.
        bsbs = {h: norm_prep(pcs[h], lo, 512) for h in (0, 1)}
        for h in (0, 1):
            norm_mult(pcs[h], bsbs[h], lo, 512, otr[h * 64:(h + 1) * 64, lo:512])

    def do_outproj(c, act_frac=2):
        # drain copies go (8-act_frac):act_frac DVE:ACT per chunk — ACT is
        # saturated by exps mid-kernel (act_frac=2); the tail chunks run when
        # exps are done, so they split evenly (act_frac=4).
        otr = otrs[c]
        n_copy = 0
        for t in range(4):
            j = 4 * c + t
            osb = streamp.tile([P, D], bf16, tag="osb")
            for half in (0, 1):
                pso = ps_big.tile([P, 512], f32, tag="psbig")
                nc.tensor.matmul(
                    pso[:], otr[:, t * P:(t + 1) * P],
                    wo_r[:, half * 512:(half + 1) * 512],
                    start=True, stop=True,
                )
                sl = slice(half * 512, (half + 1) * 512)
                if n_copy % (8 // act_frac) == 0:
                    nc.scalar.activation(osb[:, sl], pso[:], Act.Identity)
                else:
                    nc.vector.tensor_copy(osb[:, sl], pso[:])
                n_copy += 1
            nc.gpsimd.dma_start(out[j * P:(j + 1) * P, :], osb[:])

    if PHASES == "qkv":
        do_quad(1)
        do_quad(2)
        return
    if PHASES == "attn1":
        do_quad(1); do_quad(2)
        do_scores(1)
        do_av(1)
        do_outproj(1)
        return
    # wavefront: S one chunk ahead of A, two ahead of O. Within each slot the
    # consumer phases (A, O) are emitted BEFORE S(c): a score matmul stalled
    # on its PSUM rotation (waiting for an exp) then sits at the END of the
    # PE FIFO instead of head-of-line blocking the already-runnable AV and
    # out-projection matmuls. Quads 1/2 fill the early slots; do_global is
    # split around outproj(4) so its ACT-bound exp stretch has PE company.
    do_scores(1)
    do_quad(1)
    do_av(1); do_scores(7)
    do_quad(2)
    do_av(7); do_outproj(1); do_scores(2)
    do_av(2); do_outproj(7); do_scores(3)
    do_av(3); do_outproj(2); do_scores(4)
    do_av(4); do_outproj(3); do_scores(5)
    pgs = do_global_open()
    do_global_range(pgs, 0, 8)
    do_outproj(4); do_scores(0, diag_in_med=True)
    do_global_range(pgs, 8, 16)
    do_global_close(pgs)
    do_av(5); do_outproj(5); do_scores(6, diag_in_med=True)
    do_av(0); do_outproj(0, act_frac=4)
    do_av(6); do_outproj(6, act_frac=4)


def _get_nc(reps=1):
    key = ("nc", reps)
    if key not in _CACHE:
        _CACHE[key] = _build_nc(reps)
    return _CACHE[key]


def _bf16(a):
    import ml_dtypes

    return np.asarray(a, dtype=np.float32).astype(ml_dtypes.bfloat16)


def _prep_inputs(x, w_qkv, b_qkv):
    x2 = np.asarray(x, dtype=np.float32).reshape(L, D)
    xT = _bf16(np.ascontiguousarray(x2.T))
    w_qkv = np.asarray(w_qkv, dtype=np.float32)
    b_qkv = np.asarray(b_qkv, dtype=np.float32)

    def tile_w(w_slice):
        wt = w_slice.T
        return _bf16(np.ascontiguousarray(
            wt.reshape(8, P, P).transpose(1, 0, 2).reshape(P, D)
        ))

    maps = []
    for c in range(8):
        a = 2 * c * HD
        b = a + 2 * HD
        maps.append({
            "xT": xT,
            "wq": tile_w(w_qkv[a:b, :]),
            "wk": tile_w(w_qkv[D + a:D + b, :]),
            "wv": tile_w(w_qkv[2 * D + a:2 * D + b, :]),
            "bq": np.ascontiguousarray(b_qkv[a:b].reshape(P, 1)),
            "bk": np.ascontiguousarray(b_qkv[D + a:D + b].reshape(P, 1)),
        })
    return maps


def kernel(x, w_qkv, b_qkv, w_out, b_out):
    from concourse.bass_utils import run_bass_kernel_spmd

    x = np.asarray(x, dtype=np.float32)
    w_qkv = np.asarray(w_qkv, dtype=np.float32)
    b_qkv = np.asarray(b_qkv, dtype=np.float32)
    w_out = np.asarray(w_out, dtype=np.float32)
    b_out = np.asarray(b_out, dtype=np.float32)

    nc = _get_nc()
    maps = _prep_inputs(x, w_qkv, b_qkv)
    for c in range(8):
        a = 2 * c * HD
        b = a + 2 * HD
        maps[c]["wo"] = _bf16(np.ascontiguousarray(w_out[:, a:b].T))

    res = run_bass_kernel_spmd(nc, maps, core_ids=list(range(8)))

    total = res.results[0]["out"].astype(np.float32)
    for c in range(1, 8):
        total += res.results[c]["out"].astype(np.float32)
    const_row = b_qkv[2 * D:3 * D] @ w_out.T + b_out
    total += const_row[None, :]
    return total.reshape(x.shape).astype(np.float32)

